# revision 22
# baseline (speedup 1.0000x reference)
"""Trainium2 Bass kernel for nn_CholeskyResHead_68255620268805  (v2).

Reference math (per mixture component c of C=10):
    Ks = Ls @ Ls.T ; Kt = Lt @ Lt.T            (spatial 207x207, temporal 12x12)
    M  = (Ks  (x)  Kt + sig^2 I)^-1            (via eigh + explicit kron in ref)
    quad[b,c] = r_b^T M r_b                    (r = (target-mu).reshape(b, n*t))
    ll = -0.5*n*t*log(2pi) - 0.5*quad + n*Vlog + t*Ulog + log w
    nll_loss = mean_b(-logsumexp_c ll)
    mse_loss = mean(|mu-target| * mask/mean(mask)),  mask = (unscaled != 0)
    out = 0.1*nll_loss + 0.9*mse_loss

Key identity: with Ks = Us Ds Us^T, Kt = Ut Dt Ut^T,
    quad[b,c] = sum_{m,j} (Us^T R_b Ut)[m,j]^2 / (Ds[m] Dt[j] + sig^2)

v2 restructure vs v1: the tiny temporal rotation A_c = R_b @ Ut_c
(B*N*T*T*C ~ 19M MACs, ~6% of the main GEMM's work) moves to host
prep, which removes v1's entire second device GEMM stage *and* the
PSUM->SBUF eviction between the two GEMMs.  The device pipeline per
core is:

  1. Y[m, (b,j)] = Us_c^T A_c      one fp8 DoubleRow GEMM (K=207 in one
     pass), m-columns on PSUM partitions in 5 blocks of <=104, (b,j) on
     the free axis (384 cols = 32 batches x 12 eigvecs).
  2. sq = Square(Y)                ACT, fused with the PSUM->SBUF move.
  3. sqw = sq * icap[j,m]          DVE fp16 2x, icap broadcast over b
     via a stride-0 AP (j on the innermost free axis).
  4. S[slot,(b,j)] = sum_m sqw     ones-stationary matmuls accumulating
     the blocks of each slot into one PSUM row; the masked-MAE partial
     sums ride along as a 4th PSUM row (fp8 DoubleRow ones-matmul over
     host-premasked |resid|).
  5. quad[slot,b] = sum_j S        one DVE 3D reduce straight into the
     output tile; DMA out [4, 32].

Distribution: 8 cores = 4 component-groups x 2 batch-halves (same as
v1); partial components (c2, c7 split across groups) summed on host.
Host does the small eigendecompositions, A_c, the (64,10) logsumexp and
the final scalar combine; the device does all batch-sized GEMM +
reduction work.
"""

import numpy as np

B, N, T, C = 64, 207, 12, 10
NT = N * T
RHO = 0.1
LOG2PI = float(np.log(2.0 * np.pi))
NCORES = 8

G_B = 2                  # batch halves
G_C = 4                  # component groups
BH = B // G_B            # 32 batches per core
F = BH * T               # 384 free columns: (b, j), b-major
KP = 104                 # fp8 DoubleRow K partitions (2*104 >= 207)
NBLK = 5                 # m-blocks per core, each <=104 wide, zero padded

# component -> (group, slot) column assignments; (c, mlo, mhi) per slot
SLOT_DEFS = [
    [(0, 0, 207), (1, 0, 207), (2, 0, 104)],
    [(3, 0, 207), (4, 0, 207), (2, 104, 207)],   # 103 cols + 1 zero pad
    [(5, 0, 207), (6, 0, 207), (7, 0, 104)],
    [(8, 0, 207), (9, 0, 207), (7, 104, 207)],   # 103 cols + 1 zero pad
]


def _blocks_for_group(g):
    """5 (slot, c, mlo, width) m-blocks, each <=104 wide."""
    out = []
    for s, (c, mlo, mhi) in enumerate(SLOT_DEFS[g]):
        w = mhi - mlo
        while w > 0:
            bw = min(104, w)
            out.append((s, c, mlo, bw))
            mlo += bw
            w -= bw
    assert len(out) == NBLK
    return out


# t8a (fp8, [KP, 2, W8A]): [ us0 us1 A_slot0 | us2 us3 A_slot1 ]
# (each half is one DMA chain: a slot's stationary+moving land together)
US0_OFF = 0
US1_OFF = 104
A0_OFF = 208
US2_OFF = 592
US3_OFF = 696
A1_OFF = 800
W8A = 1184
# t8b (fp8, [KP, 2, W8B]): [ us4 A_slot2 | sel8 (mae one-hot, 4) mr ]
USB_OFF = 0
A2_OFF = 104
SEL8_OFF = 488
MR_OFF = 492
W8B = 880
# t16 (fp16, [KP, W16]): [ icb (5 blocks x 12 j) | sel16 (3 slots x 4) ]
ICB_OFF = 0
SEL16_OFF = 60
W16 = 72

_CACHE: dict = {}
ABLATE = None
# Tunables (A/B-tested via abbench.py / simtrace.py):
WARM_MMS = 0             # dummy PE matmuls issued during the DMA wait to
                         # flip the HAM clock gate before the real GEMMs
DMA_SPLIT = 4            # input DMA chains: 2 = one per image, 4 = per-slot
SQ_ONE = False           # single big ACT square op instead of per-slot
SQ_BLK = False           # per-block (5x) squares+mults instead of per-slot
D16_RING = "gpsimd"      # d16 chain: "first"/"last" on scalar, or "gpsimd"
B_RING = "scalar"        # ring for the t8b chains: "scalar" or "sync"


def _declare_io(nc, f32):
    import concourse.mybir as mybir

    f16 = mybir.dt.float16
    f8 = mybir.dt.float8e4
    t = {}
    t["d8a"] = nc.dram_tensor("d8a", [KP, 2, W8A], f8, kind="ExternalInput")
    t["d8b"] = nc.dram_tensor("d8b", [KP, 2, W8B], f8, kind="ExternalInput")
    t["d16"] = nc.dram_tensor("d16", [KP, W16], f16, kind="ExternalInput")
    t["oq"] = nc.dram_tensor("oq", [4, 32], f32, kind="ExternalOutput")
    return t


# static block template (same for every group): widths after padding are
# always 104, slots own blocks [0,1], [2,3], [4].
BLK_SLOT = (0, 0, 1, 1, 2)
SLOT_BLKS = ((0, 1), (2, 3), (4,))


def _emit_body(nc, tc, io):
    import concourse.mybir as mybir

    f32 = mybir.dt.float32
    f16 = mybir.dt.float16
    AF = mybir.ActivationFunctionType
    OP = mybir.AluOpType
    AX = mybir.AxisListType
    PM = mybir.MatmulPerfMode

    with (
        tc.tile_pool(name="cst", bufs=1) as cst,
        tc.tile_pool(name="ps_y", bufs=1, space="PSUM") as ps_y,
        tc.tile_pool(name="ps_s", bufs=1, space="PSUM") as ps_s,
    ):
        # ---- loads: 3 input DMA chains on separate rings ----
        t8a = cst.tile([KP, 2, W8A], mybir.dt.float8e4, tag="t8a")
        t8b = cst.tile([KP, 2, W8B], mybir.dt.float8e4, tag="t8b")
        t16 = cst.tile([KP, W16], f16, tag="t16")
        if D16_RING == "first":
            nc.scalar.dma_start(t16[:], io["d16"][:])
        elif D16_RING == "gpsimd":
            nc.gpsimd.dma_start(t16[:], io["d16"][:])
        bring = nc.sync if B_RING == "sync" else nc.scalar
        if DMA_SPLIT == 4:
            nc.sync.dma_start(t8a[:, :, 0:US2_OFF], io["d8a"][:, :, 0:US2_OFF])
            nc.sync.dma_start(t8a[:, :, US2_OFF:], io["d8a"][:, :, US2_OFF:])
            bring.dma_start(t8b[:, :, 0:SEL8_OFF], io["d8b"][:, :, 0:SEL8_OFF])
            bring.dma_start(t8b[:, :, SEL8_OFF:], io["d8b"][:, :, SEL8_OFF:])
        else:
            nc.sync.dma_start(t8a[:], io["d8a"][:])
            bring.dma_start(t8b[:], io["d8b"][:])
        if D16_RING == "last":
            nc.scalar.dma_start(t16[:], io["d16"][:])

        if WARM_MMS:
            # Dummy matmuls keep the PE busy while the input DMAs stream,
            # so the HAM clock gate reaches 8/8 before (or during) the
            # real GEMMs.  Output bank is never read.
            wsc = cst.tile([128, 512], f16, tag="wsc")
            wps = ps_y.tile([128, 512], f32, tag="wps")
            nc.vector.memset(wsc[:], 0.0)
            for _ in range(WARM_MMS):
                nc.tensor.matmul(
                    wps[:], wsc[:, 0:128], wsc[:], start=True, stop=True
                )

        ot = cst.tile([4, 32], f32, tag="ot")

        if ABLATE == "loads":
            nc.vector.tensor_scalar(
                ot[:], t8a[0:4, 0, 0:64].bitcast(f16), 0.0, None, op0=OP.mult
            )
            nc.vector.tensor_scalar(
                ot[:], t8b[0:4, 0, 0:64].bitcast(f16), 0.0, None, op0=OP.mult
            )
            nc.vector.tensor_scalar(
                ot[:], t16[0:4, 0:32], 0.0, None, op0=OP.mult
            )
            nc.sync.dma_start(io["oq"][:], ot[:])
            return

        # ---- Y = Us^T A : 5 fp8 DoubleRow matmuls, one K pass each ----
        yps = ps_y.tile([KP, NBLK, 512], f32, tag="yps")
        us_src = (
            (t8a, US0_OFF), (t8a, US1_OFF),
            (t8a, US2_OFF), (t8a, US3_OFF),
            (t8b, USB_OFF),
        )
        a_src = ((t8a, A0_OFF), (t8a, A1_OFF), (t8b, A2_OFF))
        for i in range(NBLK):
            ut, uo = us_src[i]
            at, ao = a_src[BLK_SLOT[i]]
            nc.tensor.matmul(
                yps[:, i, 0:F],
                ut[:, :, uo : uo + 104],
                at[:, :, ao : ao + F],
                start=True,
                stop=True,
                perf_mode=PM.DoubleRow,
            )

        if ABLATE == "y":
            nc.vector.tensor_copy(ot[:], yps[0:4, 0, 0:32])
            nc.sync.dma_start(io["oq"][:], ot[:])
            return

        # ---- MAE partial sums (fp8 DoubleRow one-hot matmul, row 3) ----
        # All S matmuls write the full [4, F] region (one-hot stationary
        # columns keep the other rows zero) so the output base partition
        # stays 0; they form a single PSUM accumulation group.
        sps = ps_s.tile([4, 512], f32, tag="sps")
        nc.tensor.matmul(
            sps[0:4, 0:F],
            t8b[:, :, SEL8_OFF : SEL8_OFF + 4],
            t8b[:, :, MR_OFF : MR_OFF + F],
            start=True,
            stop=False,
            perf_mode=PM.DoubleRow,
        )

        # ---- per slot: square (ACT) -> *icap (DVE) -> m-sum (PE) ----
        sq = cst.tile([KP, NBLK, F], f16, tag="sq")
        sqw = cst.tile([KP, NBLK, F], f16, tag="sqw")
        icb = t16[:, ICB_OFF : ICB_OFF + 60].rearrange(
            "p (k j) -> p k j", k=NBLK
        )
        if SQ_ONE:
            nc.scalar.activation(sq[:], yps[:, :, 0:F], AF.Square)
        groups = (
            [(BLK_SLOT[i], (i,)) for i in range(NBLK)]
            if SQ_BLK
            else list(enumerate(SLOT_BLKS))
        )
        for s, blks in groups:
            lo, hi = blks[0], blks[-1] + 1
            if not SQ_ONE:
                nc.scalar.activation(
                    sq[:, lo:hi, :], yps[:, lo:hi, 0:F], AF.Square
                )
            nc.vector.tensor_tensor(
                sqw[:, lo:hi, :].rearrange("p k (b j) -> p k b j", j=T),
                sq[:, lo:hi, :].rearrange("p k (b j) -> p k b j", j=T),
                icb[:, lo:hi, :].unsqueeze(2).broadcast_to((KP, hi - lo, BH, T)),
                op=OP.mult,
            )
            sel = t16[:, SEL16_OFF + 4 * s : SEL16_OFF + 4 * s + 4]
            for i in blks:
                nc.tensor.matmul(
                    sps[0:4, 0:F],
                    sel,
                    sqw[:, i, :],
                    start=False,
                    stop=(i == NBLK - 1),
                )

        if ABLATE == "s":
            nc.vector.tensor_copy(ot[:], sps[0:4, 0:32])
            nc.sync.dma_start(io["oq"][:], ot[:])
            return

        # ---- quad[slot, b] = sum_j S[slot, (b, j)] ; mae row rides along ----
        nc.vector.tensor_reduce(
            ot[:],
            sps[0:4, 0:F].rearrange("p (b j) -> p b j", j=T),
            axis=AX.X,
            op=OP.add,
        )
        nc.sync.dma_start(io["oq"][:], ot[:])


def _build_program():
    import concourse.bacc as bacc
    import concourse.mybir as mybir
    from concourse import tile

    f32 = mybir.dt.float32
    nc = bacc.Bacc(None, target_bir_lowering=False)
    io = _declare_io(nc, f32)
    with tile.TileContext(nc) as tc:
        _emit_body(nc, tc, io)
    nc.compile()
    return nc


def _get_program():
    if "nc" not in _CACHE:
        _CACHE["nc"] = _build_program()
    return _CACHE["nc"]


def _to_f8(x):
    import ml_dtypes

    return np.asarray(x, dtype=np.float32).astype(ml_dtypes.float8_e4m3)


def _khalves(a2d, dtype):
    """(N, X) -> (KP, 2, X) K-stacked halves with zero pad row 207."""
    out = np.zeros((KP, 2, a2d.shape[1]), dtype=dtype)
    out[:, 0, :] = a2d[0:KP]
    out[0 : N - KP, 1, :] = a2d[KP:N]
    return out


def _host_prep(mu, target, unscaled_target, w, sigma, L_spatial, L_temporal):
    """Builds per-core input maps and the host-side ll constants."""
    import ml_dtypes

    f = np.float32
    h = np.float16
    f8 = ml_dtypes.float8_e4m3
    mu = np.asarray(mu, dtype=f)
    target = np.asarray(target, dtype=f)
    unscaled_target = np.asarray(unscaled_target, dtype=f)
    Ls = np.asarray(L_spatial, dtype=np.float64)
    Lt = np.asarray(L_temporal, dtype=np.float64)

    Ks = Ls @ np.transpose(Ls, (0, 2, 1))
    Kt = Lt @ np.transpose(Lt, (0, 2, 1))
    Ds, Us = np.linalg.eigh(Ks)                   # (C, N), (C, N, N)
    Dt, Ut = np.linalg.eigh(Kt)                   # (C, T), (C, T, T)
    sig2 = np.asarray(sigma, dtype=np.float64) ** 2
    icap = 1.0 / (Dt[:, :, None] * Ds[:, None, :] + sig2[:, None, None])  # (C,T,N)

    resid = (target - mu)                         # (B, N, T)
    masku = unscaled_target != 0
    sum_cnt = float(masku.sum())
    mr = np.abs(resid) * masku                    # masked |resid|
    mr8 = _khalves(
        _to_f8(mr.transpose(1, 0, 2).reshape(N, B * T)), f8
    )                                             # (KP, 2, 768), cols (b, t)

    # A_c[n, (b, j)] = sum_t resid[b, n, t] * Ut_c[t, j]   (cols b-major)
    A = np.einsum("bnt,ctj->cnbj", resid.astype(np.float64), Ut)
    A8 = [_khalves(_to_f8(A[c].reshape(N, B * T)), f8) for c in range(C)]
    Us8 = [_to_f8(Us[c]) for c in range(C)]

    Ulog = np.sum(np.log(np.einsum("cii->ci", Ls)), axis=1)
    Vlog = np.sum(np.log(np.einsum("cii->ci", Lt)), axis=1)
    logw = np.log(np.asarray(w, dtype=np.float64)[..., 0])
    m2_full = (
        -0.5 * NT * LOG2PI + N * Vlog[None, :] + T * Ulog[None, :] + logw
    ).astype(f)                                   # (B, C)

    in_maps = []
    for k in range(NCORES):
        g, hh = k // G_B, k % G_B
        bsl = slice(hh * F, (hh + 1) * F)
        blocks = _blocks_for_group(g)

        d8a = np.zeros((KP, 2, W8A), dtype=f8)
        d8b = np.zeros((KP, 2, W8B), dtype=f8)
        d16 = np.zeros((KP, W16), dtype=h)

        slot_cs = [sd[0] for sd in SLOT_DEFS[g]]
        d8a[:, :, A0_OFF : A0_OFF + F] = A8[slot_cs[0]][:, :, bsl]
        d8a[:, :, A1_OFF : A1_OFF + F] = A8[slot_cs[1]][:, :, bsl]
        d8b[:, :, A2_OFF : A2_OFF + F] = A8[slot_cs[2]][:, :, bsl]
        d8b[:, :, MR_OFF : MR_OFF + F] = mr8[:, :, bsl]
        # one-hot stationaries: mae -> row 3 (fp8, row 207 pad = 0),
        # slot s -> row s (fp16, all 104 partitions; padded m rows hold
        # zero data so an all-ones column is safe)
        sel8 = np.zeros((N, 4), dtype=f)
        sel8[:, 3] = 1.0
        d8b[:, :, SEL8_OFF : SEL8_OFF + 4] = _khalves(sel8, f8)
        for s in range(3):
            d16[:, SEL16_OFF + 4 * s + s] = np.float16(1.0)

        us_cols = (
            (d8a, US0_OFF), (d8a, US1_OFF), (d8a, US2_OFF),
            (d8a, US3_OFF), (d8b, USB_OFF),
        )
        for i, (s, c, mlo, bw) in enumerate(blocks):
            dst, col = us_cols[i]
            dst[:, :, col : col + bw] = _khalves(Us8[c][:, mlo : mlo + bw], f8)
            # icb[m_local, block, j] = icap[c][j, mlo + m_local]
            d16[0:bw, ICB_OFF + i * T : ICB_OFF + (i + 1) * T] = (
                icap[c][:, mlo : mlo + bw].T.astype(h)
            )

        in_maps.append({"d8a": d8a, "d8b": d8b, "d16": d16})
    return in_maps, m2_full, sum_cnt


def _host_final(results, m2_full, sum_cnt):
    quad = np.zeros((B, C), dtype=np.float32)
    for k in range(NCORES):
        g, h = k // G_B, k % G_B
        oq = results[k]["oq"]
        for s, (c, _mlo, _mhi) in enumerate(SLOT_DEFS[g]):
            quad[h * BH : (h + 1) * BH, c] += oq[s, :]
    sum_abs = float(results[0]["oq"][3].sum()) + float(results[1]["oq"][3].sum())

    ll = m2_full - np.float32(0.5) * quad
    mx = ll.max(axis=1, keepdims=True)
    lse = np.log(np.exp(ll - mx).sum(axis=1, keepdims=True, dtype=np.float32)) + mx
    nll_loss = -np.float32(lse.sum()) / np.float32(B)
    mse_loss = np.float32(sum_abs) / np.float32(sum_cnt)
    out = np.float32(RHO) * nll_loss + np.float32(1.0 - RHO) * mse_loss
    return np.asarray(out, dtype=np.float32)


def kernel(**inputs) -> np.ndarray:
    from concourse.bass_utils import run_bass_kernel_spmd

    nc = _get_program()
    in_maps, m2_full, sum_cnt = _host_prep(
        inputs["mu"],
        inputs["target"],
        inputs["unscaled_target"],
        inputs["w"],
        inputs["sigma"],
        inputs["L_spatial"],
        inputs["L_temporal"],
    )
    res = run_bass_kernel_spmd(nc, in_maps, list(range(NCORES))).results
    return _host_final(res, m2_full, sum_cnt)


# revision 44
# speedup vs baseline: 1.0786x; 1.0786x over previous
"""Trainium2 Bass kernel for nn_CholeskyResHead_68255620268805  (v2).

Reference math (per mixture component c of C=10):
    Ks = Ls @ Ls.T ; Kt = Lt @ Lt.T            (spatial 207x207, temporal 12x12)
    M  = (Ks  (x)  Kt + sig^2 I)^-1            (via eigh + explicit kron in ref)
    quad[b,c] = r_b^T M r_b                    (r = (target-mu).reshape(b, n*t))
    ll = -0.5*n*t*log(2pi) - 0.5*quad + n*Vlog + t*Ulog + log w
    nll_loss = mean_b(-logsumexp_c ll)
    mse_loss = mean(|mu-target| * mask/mean(mask)),  mask = (unscaled != 0)
    out = 0.1*nll_loss + 0.9*mse_loss

Key identity: with Ks = Us Ds Us^T, Kt = Ut Dt Ut^T,
    quad[b,c] = sum_{m,j} (Us^T R_b Ut)[m,j]^2 / (Ds[m] Dt[j] + sig^2)

v2 restructure vs v1: the tiny temporal rotation A_c = R_b @ Ut_c
(B*N*T*T*C ~ 19M MACs, ~6% of the main GEMM's work) moves to host
prep, which removes v1's entire second device GEMM stage *and* the
PSUM->SBUF eviction between the two GEMMs.  The device pipeline per
core is:

  0. Input DMA: d16 (icap+selectors, tiny) first on the scalar ring so
     it never gates the DVE stage; d8a split into two per-slot chains
     on the sync ring (slot-0 compute starts before slot-1 data lands);
     d8b on the scalar ring.  A handful of dummy matmuls on scratch
     SBUF run during the wait purely to flip the PE HAM clock-gate to
     8/8 before the real GEMMs (measured ~0.4us win).
  1. Y[m, (b,j)] = Us_c^T A_c      one fp8 DoubleRow GEMM (K=207 in one
     pass), m-columns on PSUM partitions in 5 blocks of <=104, (b,j) on
     the free axis (384 cols = 32 batches x 12 eigvecs).
  2. sq = Square(Y)                ACT, fused with the PSUM->SBUF move.
  3. sqw = sq * icap[j,m]          DVE fp16 2x, icap broadcast over b
     via a stride-0 AP (j on the innermost free axis).
  4. S[slot,(b,j)] = sum_m sqw     one-hot-stationary matmuls (all
     writing the full [4,F] PSUM region at base partition 0, one
     accumulation group); the masked-MAE partial sums ride along as
     row 3 (fp8 DoubleRow one-hot matmul over host-premasked |resid|).
  5. quad[slot,b] = sum_j S        one DVE 3D reduce straight into the
     output tile; DMA out [4, 32].

Stages 2-4 run per-slot so ACT/DVE/PE pipeline across slots; the small
single-block slot goes last to shorten the tail.  Measured per-
iteration (For_i steady state, incl. ~2.3us loop overhead): ~12.4us vs
24.7us for v1, with rel err 5.4e-4 (better than v1's 8.1e-4 since Ut
is folded in f64 on the host).

Distribution: 8 cores = 4 component-groups x 2 batch-halves (same as
v1); partial components (c2, c7 split across groups) summed on host.
Host does the small eigendecompositions, A_c, the (64,10) logsumexp and
the final scalar combine; the device does all batch-sized GEMM +
reduction work.
"""

import numpy as np

B, N, T, C = 64, 207, 12, 10
NT = N * T
RHO = 0.1
LOG2PI = float(np.log(2.0 * np.pi))
NCORES = 8

G_B = 2                  # batch halves
G_C = 4                  # component groups
BH = B // G_B            # 32 batches per core
F = BH * T               # 384 free columns: (b, j), b-major
KP = 104                 # fp8 DoubleRow K partitions (2*104 >= 207)
NBLK = 5                 # m-blocks per core, each <=104 wide, zero padded

# component -> (group, slot) column assignments; (c, mlo, mhi) per slot
SLOT_DEFS = [
    [(0, 0, 207), (1, 0, 207), (2, 0, 104)],
    [(3, 0, 207), (4, 0, 207), (2, 104, 207)],   # 103 cols + 1 zero pad
    [(5, 0, 207), (6, 0, 207), (7, 0, 104)],
    [(8, 0, 207), (9, 0, 207), (7, 104, 207)],   # 103 cols + 1 zero pad
]


def _blocks_for_group(g):
    """5 (slot, c, mlo, width) m-blocks, each <=104 wide."""
    out = []
    for s, (c, mlo, mhi) in enumerate(SLOT_DEFS[g]):
        w = mhi - mlo
        while w > 0:
            bw = min(104, w)
            out.append((s, c, mlo, bw))
            mlo += bw
            w -= bw
    assert len(out) == NBLK
    return out


# t8a (fp8, [KP, 2, W8A]): [ us0 us1 A_slot0 | us2 us3 A_slot1 | t16 bytes ]
# (first two thirds are one DMA chain each under DMA_SPLIT=4)
US0_OFF = 0
US1_OFF = 104
A0_OFF = 208
US2_OFF = 592
US3_OFF = 696
A1_OFF = 800
W8A = 1184               # +2*W16 when D16_RING == "merged_a"
# t8b (fp8, [KP, 2, W8B]): [ us4 A_slot2 | sel8 (mae one-hot, 4) mr | t16 bytes ]
USB_OFF = 0
A2_OFF = 104
SEL8_OFF = 488
MR_OFF = 492
W8B = 880                # +2*W16 when D16_RING == "merged_b"
# t16 (fp16, [KP, W16]): [ icb (5 blocks x 12 j) | sel16 (3 slots x 4) ]
ICB_OFF = 0
SEL16_OFF = 60
W16 = 72


def _w8():
    """(w8a, w8b) widths; the merged variants append the fp16 payload."""
    return (
        W8A + (2 * W16 if D16_RING == "merged_a" else 0),
        W8B + (2 * W16 if D16_RING == "merged_b" else 0),
    )

_CACHE: dict = {}
ABLATE = None
# Tunables (A/B-tested via abbench.py / simtrace.py):
WARM_MMS = 6             # dummy PE matmuls issued during the DMA wait to
                         # flip the HAM clock gate before the real GEMMs
DMA_SPLIT = 3            # input DMA chains: 2 = one per image, 4 = per-slot
SQ_ONE = False           # single big ACT square op instead of per-slot
SQ_BLK = False           # per-block (5x) squares+mults instead of per-slot
SQ_G12 = False           # group slot1+slot2 squares into one ACT op
D16_RING = "first"       # d16 chain: "first"/"last" on scalar, or "gpsimd"
B_RING = "scalar"        # ring for the t8b chains: "scalar" or "sync"
ONE_PSUM_POOL = True     # allocate sps in the same PSUM pool as yps


def _declare_io(nc, f32):
    import concourse.mybir as mybir

    f16 = mybir.dt.float16
    f8 = mybir.dt.float8e4
    t = {}
    w8a, w8b = _w8()
    t["d8a"] = nc.dram_tensor("d8a", [KP, 2, w8a], f8, kind="ExternalInput")
    t["d8b"] = nc.dram_tensor("d8b", [KP, 2, w8b], f8, kind="ExternalInput")
    if not D16_RING.startswith("merged"):
        t["d16"] = nc.dram_tensor("d16", [KP, W16], f16, kind="ExternalInput")
    t["oq"] = nc.dram_tensor("oq", [4, 32], f32, kind="ExternalOutput")
    return t


# static block template (same for every group): widths after padding are
# always 104, slots own blocks [0,1], [2,3], [4].
BLK_SLOT = (0, 0, 1, 1, 2)
SLOT_BLKS = ((0, 1), (2, 3), (4,))


def _emit_body(nc, tc, io):
    import concourse.mybir as mybir

    f32 = mybir.dt.float32
    f16 = mybir.dt.float16
    AF = mybir.ActivationFunctionType
    OP = mybir.AluOpType
    AX = mybir.AxisListType
    PM = mybir.MatmulPerfMode

    with (
        tc.tile_pool(name="cst", bufs=1) as cst,
        tc.tile_pool(name="ps_y", bufs=1, space="PSUM") as ps_y,
        tc.tile_pool(name="ps_s", bufs=1, space="PSUM") as ps_s_pool,
    ):
        ps_s = ps_y if ONE_PSUM_POOL else ps_s_pool
        # ---- loads: 3 input DMA chains on separate rings ----
        w8a, w8b = _w8()
        t8a = cst.tile([KP, 2, w8a], mybir.dt.float8e4, tag="t8a")
        t8b = cst.tile([KP, 2, w8b], mybir.dt.float8e4, tag="t8b")
        if D16_RING == "merged_a":
            t16 = t8a[:, 0, W8A : W8A + 2 * W16].bitcast(f16)
        elif D16_RING == "merged_b":
            t16 = t8b[:, 0, W8B : W8B + 2 * W16].bitcast(f16)
        else:
            t16 = cst.tile([KP, W16], f16, tag="t16")
        if D16_RING == "first":
            nc.scalar.dma_start(t16[:], io["d16"][:])
        elif D16_RING == "gpsimd":
            nc.gpsimd.dma_start(t16[:], io["d16"][:])
        bring = nc.sync if B_RING == "sync" else nc.scalar
        if DMA_SPLIT == 4:
            nc.sync.dma_start(t8a[:, :, 0:US2_OFF], io["d8a"][:, :, 0:US2_OFF])
            nc.sync.dma_start(t8a[:, :, US2_OFF:], io["d8a"][:, :, US2_OFF:])
            bring.dma_start(t8b[:, :, 0:SEL8_OFF], io["d8b"][:, :, 0:SEL8_OFF])
            bring.dma_start(t8b[:, :, SEL8_OFF:], io["d8b"][:, :, SEL8_OFF:])
        elif DMA_SPLIT == 3:
            nc.sync.dma_start(t8a[:, :, 0:US2_OFF], io["d8a"][:, :, 0:US2_OFF])
            nc.sync.dma_start(t8a[:, :, US2_OFF:], io["d8a"][:, :, US2_OFF:])
            bring.dma_start(t8b[:], io["d8b"][:])
        else:
            nc.sync.dma_start(t8a[:], io["d8a"][:])
            bring.dma_start(t8b[:], io["d8b"][:])
        if D16_RING == "last":
            nc.scalar.dma_start(t16[:], io["d16"][:])

        if WARM_MMS:
            # Dummy matmuls keep the PE busy while the input DMAs stream,
            # so the HAM clock gate reaches 8/8 before (or during) the
            # real GEMMs.  Output bank is never read.
            wsc = cst.tile([128, 512], f16, tag="wsc")
            wps = ps_y.tile([128, 512], f32, tag="wps")
            nc.vector.memset(wsc[:], 0.0)
            for _ in range(WARM_MMS):
                nc.tensor.matmul(
                    wps[:], wsc[:, 0:128], wsc[:], start=True, stop=True
                )

        ot = cst.tile([4, 32], f32, tag="ot")

        if ABLATE == "loads":
            nc.vector.tensor_scalar(
                ot[:], t8a[0:4, 0, 0:64].bitcast(f16), 0.0, None, op0=OP.mult
            )
            nc.vector.tensor_scalar(
                ot[:], t8b[0:4, 0, 0:64].bitcast(f16), 0.0, None, op0=OP.mult
            )
            nc.vector.tensor_scalar(
                ot[:], t16[0:4, 0:32], 0.0, None, op0=OP.mult
            )
            nc.sync.dma_start(io["oq"][:], ot[:])
            return

        # ---- Y = Us^T A : 5 fp8 DoubleRow matmuls, one K pass each ----
        yps = ps_y.tile([KP, NBLK, 512], f32, tag="yps")
        us_src = (
            (t8a, US0_OFF), (t8a, US1_OFF),
            (t8a, US2_OFF), (t8a, US3_OFF),
            (t8b, USB_OFF),
        )
        a_src = ((t8a, A0_OFF), (t8a, A1_OFF), (t8b, A2_OFF))
        for i in range(NBLK):
            ut, uo = us_src[i]
            at, ao = a_src[BLK_SLOT[i]]
            nc.tensor.matmul(
                yps[:, i, 0:F],
                ut[:, :, uo : uo + 104],
                at[:, :, ao : ao + F],
                start=True,
                stop=True,
                perf_mode=PM.DoubleRow,
            )

        if ABLATE == "y":
            nc.vector.tensor_copy(ot[:], yps[0:4, 0, 0:32])
            nc.sync.dma_start(io["oq"][:], ot[:])
            return

        # ---- MAE partial sums (fp8 DoubleRow one-hot matmul, row 3) ----
        # All S matmuls write the full [4, F] region (one-hot stationary
        # columns keep the other rows zero) so the output base partition
        # stays 0; they form a single PSUM accumulation group.
        sps = ps_s.tile([4, 512], f32, tag="sps")
        nc.tensor.matmul(
            sps[0:4, 0:F],
            t8b[:, :, SEL8_OFF : SEL8_OFF + 4],
            t8b[:, :, MR_OFF : MR_OFF + F],
            start=True,
            stop=False,
            perf_mode=PM.DoubleRow,
        )

        # ---- per slot: square (ACT) -> *icap (DVE) -> m-sum (PE) ----
        sq = cst.tile([KP, NBLK, F], f16, tag="sq")
        sqw = cst.tile([KP, NBLK, F], f16, tag="sqw")
        icb = t16[:, ICB_OFF : ICB_OFF + 60].rearrange(
            "p (k j) -> p k j", k=NBLK
        )
        if SQ_ONE:
            nc.scalar.activation(sq[:], yps[:, :, 0:F], AF.Square)
        elif SQ_G12:
            nc.scalar.activation(sq[:, 0:2, :], yps[:, 0:2, 0:F], AF.Square)
            nc.scalar.activation(sq[:, 2:5, :], yps[:, 2:5, 0:F], AF.Square)
        groups = (
            [(BLK_SLOT[i], (i,)) for i in range(NBLK)]
            if SQ_BLK
            else list(enumerate(SLOT_BLKS))
        )
        for s, blks in groups:
            lo, hi = blks[0], blks[-1] + 1
            if not (SQ_ONE or SQ_G12):
                nc.scalar.activation(
                    sq[:, lo:hi, :], yps[:, lo:hi, 0:F], AF.Square
                )
            nc.vector.tensor_tensor(
                sqw[:, lo:hi, :].rearrange("p k (b j) -> p k b j", j=T),
                sq[:, lo:hi, :].rearrange("p k (b j) -> p k b j", j=T),
                icb[:, lo:hi, :].unsqueeze(2).broadcast_to((KP, hi - lo, BH, T)),
                op=OP.mult,
            )
            sel = t16[:, SEL16_OFF + 4 * s : SEL16_OFF + 4 * s + 4]
            for i in blks:
                nc.tensor.matmul(
                    sps[0:4, 0:F],
                    sel,
                    sqw[:, i, :],
                    start=False,
                    stop=(i == NBLK - 1),
                )

        if ABLATE == "s":
            nc.vector.tensor_copy(ot[:], sps[0:4, 0:32])
            nc.sync.dma_start(io["oq"][:], ot[:])
            return

        # ---- quad[slot, b] = sum_j S[slot, (b, j)] ; mae row rides along ----
        nc.vector.tensor_reduce(
            ot[:],
            sps[0:4, 0:F].rearrange("p (b j) -> p b j", j=T),
            axis=AX.X,
            op=OP.add,
        )
        nc.sync.dma_start(io["oq"][:], ot[:])


def _build_program():
    import concourse.bacc as bacc
    import concourse.mybir as mybir
    from concourse import tile

    f32 = mybir.dt.float32
    nc = bacc.Bacc(None, target_bir_lowering=False)
    io = _declare_io(nc, f32)
    with tile.TileContext(nc) as tc:
        _emit_body(nc, tc, io)
    nc.compile()
    return nc


def _get_program():
    if "nc" not in _CACHE:
        _CACHE["nc"] = _build_program()
    return _CACHE["nc"]


def _to_f8(x):
    import ml_dtypes

    return np.asarray(x, dtype=np.float32).astype(ml_dtypes.float8_e4m3)


def _khalves(a2d, dtype):
    """(N, X) -> (KP, 2, X) K-stacked halves with zero pad row 207."""
    out = np.zeros((KP, 2, a2d.shape[1]), dtype=dtype)
    out[:, 0, :] = a2d[0:KP]
    out[0 : N - KP, 1, :] = a2d[KP:N]
    return out


def _host_prep(mu, target, unscaled_target, w, sigma, L_spatial, L_temporal):
    """Builds per-core input maps and the host-side ll constants."""
    import ml_dtypes

    f = np.float32
    h = np.float16
    f8 = ml_dtypes.float8_e4m3
    mu = np.asarray(mu, dtype=f)
    target = np.asarray(target, dtype=f)
    unscaled_target = np.asarray(unscaled_target, dtype=f)
    Ls = np.asarray(L_spatial, dtype=np.float64)
    Lt = np.asarray(L_temporal, dtype=np.float64)

    Ks = Ls @ np.transpose(Ls, (0, 2, 1))
    Kt = Lt @ np.transpose(Lt, (0, 2, 1))
    Ds, Us = np.linalg.eigh(Ks)                   # (C, N), (C, N, N)
    Dt, Ut = np.linalg.eigh(Kt)                   # (C, T), (C, T, T)
    sig2 = np.asarray(sigma, dtype=np.float64) ** 2
    icap = 1.0 / (Dt[:, :, None] * Ds[:, None, :] + sig2[:, None, None])  # (C,T,N)

    resid = (target - mu)                         # (B, N, T)
    masku = unscaled_target != 0
    sum_cnt = float(masku.sum())
    mr = np.abs(resid) * masku                    # masked |resid|
    mr8 = _khalves(
        _to_f8(mr.transpose(1, 0, 2).reshape(N, B * T)), f8
    )                                             # (KP, 2, 768), cols (b, t)

    # A_c[n, (b, j)] = sum_t resid[b, n, t] * Ut_c[t, j]   (cols b-major)
    A = np.einsum("bnt,ctj->cnbj", resid.astype(np.float64), Ut)
    A8 = [_khalves(_to_f8(A[c].reshape(N, B * T)), f8) for c in range(C)]
    Us8 = [_to_f8(Us[c]) for c in range(C)]

    Ulog = np.sum(np.log(np.einsum("cii->ci", Ls)), axis=1)
    Vlog = np.sum(np.log(np.einsum("cii->ci", Lt)), axis=1)
    logw = np.log(np.asarray(w, dtype=np.float64)[..., 0])
    m2_full = (
        -0.5 * NT * LOG2PI + N * Vlog[None, :] + T * Ulog[None, :] + logw
    ).astype(f)                                   # (B, C)

    in_maps = []
    for k in range(NCORES):
        g, hh = k // G_B, k % G_B
        bsl = slice(hh * F, (hh + 1) * F)
        blocks = _blocks_for_group(g)

        w8a, w8b = _w8()
        d8a = np.zeros((KP, 2, w8a), dtype=f8)
        d8b = np.zeros((KP, 2, w8b), dtype=f8)
        d16 = np.zeros((KP, W16), dtype=h)

        slot_cs = [sd[0] for sd in SLOT_DEFS[g]]
        d8a[:, :, A0_OFF : A0_OFF + F] = A8[slot_cs[0]][:, :, bsl]
        d8a[:, :, A1_OFF : A1_OFF + F] = A8[slot_cs[1]][:, :, bsl]
        d8b[:, :, A2_OFF : A2_OFF + F] = A8[slot_cs[2]][:, :, bsl]
        d8b[:, :, MR_OFF : MR_OFF + F] = mr8[:, :, bsl]
        # one-hot stationaries: mae -> row 3 (fp8, row 207 pad = 0),
        # slot s -> row s (fp16, all 104 partitions; padded m rows hold
        # zero data so an all-ones column is safe)
        sel8 = np.zeros((N, 4), dtype=f)
        sel8[:, 3] = 1.0
        d8b[:, :, SEL8_OFF : SEL8_OFF + 4] = _khalves(sel8, f8)
        for s in range(3):
            d16[:, SEL16_OFF + 4 * s + s] = np.float16(1.0)

        us_cols = (
            (d8a, US0_OFF), (d8a, US1_OFF), (d8a, US2_OFF),
            (d8a, US3_OFF), (d8b, USB_OFF),
        )
        for i, (s, c, mlo, bw) in enumerate(blocks):
            dst, col = us_cols[i]
            dst[:, :, col : col + bw] = _khalves(Us8[c][:, mlo : mlo + bw], f8)
            # icb[m_local, block, j] = icap[c][j, mlo + m_local]
            d16[0:bw, ICB_OFF + i * T : ICB_OFF + (i + 1) * T] = (
                icap[c][:, mlo : mlo + bw].T.astype(h)
            )

        if D16_RING == "merged_a":
            d8a[:, 0, W8A : W8A + 2 * W16] = d16.view(f8)
            in_maps.append({"d8a": d8a, "d8b": d8b})
        elif D16_RING == "merged_b":
            d8b[:, 0, W8B : W8B + 2 * W16] = d16.view(f8)
            in_maps.append({"d8a": d8a, "d8b": d8b})
        else:
            in_maps.append({"d8a": d8a, "d8b": d8b, "d16": d16})
    return in_maps, m2_full, sum_cnt


def _host_final(results, m2_full, sum_cnt):
    quad = np.zeros((B, C), dtype=np.float32)
    for k in range(NCORES):
        g, h = k // G_B, k % G_B
        oq = results[k]["oq"]
        for s, (c, _mlo, _mhi) in enumerate(SLOT_DEFS[g]):
            quad[h * BH : (h + 1) * BH, c] += oq[s, :]
    sum_abs = float(results[0]["oq"][3].sum()) + float(results[1]["oq"][3].sum())

    ll = m2_full - np.float32(0.5) * quad
    mx = ll.max(axis=1, keepdims=True)
    lse = np.log(np.exp(ll - mx).sum(axis=1, keepdims=True, dtype=np.float32)) + mx
    nll_loss = -np.float32(lse.sum()) / np.float32(B)
    mse_loss = np.float32(sum_abs) / np.float32(sum_cnt)
    out = np.float32(RHO) * nll_loss + np.float32(1.0 - RHO) * mse_loss
    return np.asarray(out, dtype=np.float32)


def kernel(**inputs) -> np.ndarray:
    from concourse.bass_utils import run_bass_kernel_spmd

    nc = _get_program()
    in_maps, m2_full, sum_cnt = _host_prep(
        inputs["mu"],
        inputs["target"],
        inputs["unscaled_target"],
        inputs["w"],
        inputs["sigma"],
        inputs["L_spatial"],
        inputs["L_temporal"],
    )
    res = run_bass_kernel_spmd(nc, in_maps, list(range(NCORES))).results
    return _host_final(res, m2_full, sum_cnt)
